# revision 1
# baseline (speedup 1.0000x reference)
"""CapsuleLayer (dynamic routing) Trainium2 kernel — 8 NeuronCores, SPMD.

Strategy: shard the input-capsule axis IC=9216 across 8 cores (1152 each).
Per core, the weight shard (2.95 MB bf16) and both x layouts (2.4 MB bf16)
stay resident in SBUF, so u_hat ([64,9216,10,16] = 377 MB fp32) is never
materialized in HBM — it is recomputed on the tensor engine as needed.

Per routing iteration (3 total, unrolled):
  s~_j   = sum_i exp(b_ij) * u_hat[b,i,j,s]   -> per-core partial via 72
           PSUM-accumulated matmuls over K=(128 i's) x (8 u's)
  Z_j    = sum_i exp(b_ij)                     -> softmax normalizer partial
  ONE AllReduce carries [s~ partial (64x160) ; Z partial] (41.6 KB fp32);
  normalization s = s~/Z commutes with the sum over i, so softmax needs no
  separate collective.  v = squash(s) is then computed identically on every
  core.  The agreement update u_vj = mean_b <u_hat, v> is purely local to
  the core's i-shard:
  T'[i,u,s,j] = sum_b x[b,u,i] * (v[b,j,s]/B)  (72 K=64 matmuls)
  u_vj[i,j]   = sum_{u,s} W[i,j,s,u] * T'      (DVE multiply + add-tree)
Iteration 1 uses the exact uniform softmax c=1/IC (b=0), iteration 3 skips
the dead agreement update.  b stays in [-0.04, 0.04] so exp() without
max-subtraction is exact.  All matmuls run in bf16 with fp32 PSUM
accumulation; validated end-to-end rel err ~4e-3 vs the fp32 reference.

Scheduling notes: the per-i-tile agreement pipeline fuses the b update,
exp, and (in the next weighted-sum phase) the exp(b)*W scaling so the
vector engine streams tile-by-tile; the u-sum uses a pairwise bf16 add
tree (tensor_tensor has a 2x uop, tensor_reduce is 1x-only); dummy
gpsimd-paced matmul chains + a dense burst keep the PE HAM un-throttled
across each AllReduce wait; scalar-engine activation tables (Sqrt/Exp)
are preloaded off the critical path.  The one-time collectives-firmware
boot (~45-70us, starts at a fixed ~21us into every NEFF execution)
dominates the first AllReduce and is outside kernel control.
Measured: ~175-205us HW exec (boot variance), rel err 4.1e-3.
"""

import numpy as np
import ml_dtypes

B, IU, IC, NU, US = 64, 8, 9216, 10, 16
N_CORES = 8
S = IC // N_CORES        # 1152 i's per core
M9 = S // 128            # 9 i-tiles of 128
SJ = US * NU             # 160
BF16 = ml_dtypes.bfloat16

_CACHE = {}


def _split_multi_waits(nc):
    """The walrus build in this image rejects instructions carrying more than
    one semaphore wait.  Split: for every instruction with k>1 waits, emit
    k-1 standalone wait-only EventSemaphore instructions on the same engine
    immediately before it (same ordering semantics: the engine blocks on each
    wait sequentially)."""
    import copy

    import bass_rust

    template = None
    for f in nc.m.functions:
        for blk in f.blocks:
            for inst in blk.instructions:
                if type(inst).__name__ == "InstEventSemaphore":
                    template = inst
                    break
            if template is not None:
                break
    assert template is not None, "no EventSemaphore template found"

    n = 0
    for f in nc.m.functions:
        for blk in f.blocks:
            out = []
            changed = False
            for inst in blk.instructions:
                si = inst.sync_info
                if si is not None and si.on_wait and len(si.on_wait) > 1:
                    waits = list(si.on_wait)
                    for w in waits[:-1]:
                        c = copy.deepcopy(template)
                        c.name = f"split_wait_{n}"
                        n += 1
                        c.engine = inst.engine
                        c.sync_info = bass_rust.SyncInfo(on_wait=[w], on_update=[])
                        out.append(c)
                    si.on_wait = [waits[-1]]
                    changed = True
                out.append(inst)
            if changed:
                blk.instructions = out


def _build_program():
    from concourse import bass, tile, mybir

    f32 = mybir.dt.float32
    bf16 = mybir.dt.bfloat16
    MUL = mybir.AluOpType.mult
    ADD = mybir.AluOpType.add

    nc = bass.Bass(
        "TRN2", target_bir_lowering=False, debug=False, num_devices=N_CORES
    )
    wa_in = nc.dram_tensor("wa", [128, M9, IU * US, NU], bf16, kind="ExternalInput").ap()
    xc_in = nc.dram_tensor("xc", [128, M9, IU, B], bf16, kind="ExternalInput").ap()
    xt_in = nc.dram_tensor("xt", [B, IU, S], bf16, kind="ExternalInput").ap()
    wb_in = nc.dram_tensor("wb", [128, M9, IU, SJ], bf16, kind="ExternalInput").ap()
    y_out = nc.dram_tensor("y", [B, NU, US], f32, kind="ExternalOutput").ap()

    with tile.TileContext(nc) as tc:
        with (
            tc.tile_pool(name="const", bufs=1) as cp,
            tc.tile_pool(name="work", bufs=8) as wp,
            tc.tile_pool(name="psum_s", bufs=1, space="PSUM") as pps,
            tc.tile_pool(name="psum_t", bufs=3, space="PSUM") as ppt,
            tc.tile_pool(name="psum_z", bufs=1, space="PSUM") as ppz,
            tc.tile_pool(name="dram", bufs=1, space="DRAM") as dp,
        ):
            # ---- resident tensors ----
            wa = cp.tile([128, M9, IU * US, NU], bf16, tag="wa")
            cw = cp.tile([128, M9, IU * US, NU], bf16, tag="cw")
            xc = cp.tile([128, M9, IU, B], bf16, tag="xc")
            xt = cp.tile([B, IU, S], bf16, tag="xt")
            wb2 = cp.tile([128, M9, IU, SJ], bf16, tag="wb2")
            ones = cp.tile([128, 1], f32, tag="ones")
            ones1 = cp.tile([1, B], f32, tag="ones1")
            zrow = cp.tile([1, SJ], f32, tag="zrow")
            b64 = cp.tile([B, 1], f32, tag="b64")
            tl1 = cp.tile([1, 2], f32, tag="tl1")
            tl2 = cp.tile([1, 2], f32, tag="tl2")
            b1c = cp.tile([B, 1], f32, tag="b1c")
            warm = cp.tile([128, 128], bf16, tag="warm")
            gbig = cp.tile([128, 512], bf16, tag="gbig")
            b_acc = cp.tile([128, M9, NU], f32, tag="bacc")
            e128 = cp.tile([128, M9, NU], bf16, tag="e128")
            uv = cp.tile([128, M9, NU], f32, tag="uv")
            zred = cp.tile([128, NU], f32, tag="zred")

            nc.sync.dma_start(out=wa[:], in_=wa_in[:])
            nc.sync.dma_start(out=xc[:], in_=xc_in[:])
            nc.sync.dma_start(out=xt[:], in_=xt_in[:])
            nc.sync.dma_start(out=wb2[:], in_=wb_in[:])
            nc.vector.memset(ones[:], 1.0)
            nc.vector.memset(ones1[:], 1.0)
            nc.vector.memset(zrow[:], 0.0)
            nc.vector.memset(b64[:], float(B))
            nc.vector.memset(tl1[:], 1.0)
            nc.scalar.sqrt(tl2[:], tl1[:])
            nc.vector.memset(b1c[:], 1.0)
            nc.vector.memset(warm[:], 0)
            nc.vector.memset(gbig[:], 0)

            # PE warm-up during the input DMAs (HAM un-throttle needs ~3.5us
            # of sustained matmul activity).
            pw = ppz.tile([128, 128], f32, tag="pz")
            for _ in range(40):
                nc.tensor.matmul(pw[:], warm[:], warm[:], start=True, stop=True)

            ar_bufs = []
            for it in range(3):
                ar_in = dp.tile([65, SJ], f32, tag=f"arin{it}")
                ar_out = dp.tile([65, SJ], f32, tag=f"arout{it}")
                ar_bufs.append((ar_in, ar_out))

            for it in range(3):
                ar_in, ar_out = ar_bufs[it]
                # ---- weighted-sum matmuls: s~ partial [64, (s,j)] ----
                rhs_src = wa if it == 0 else cw
                ps = pps.tile([B, US, NU], f32, tag="ps")
                n_mm = M9 * IU
                k = 0
                for m in range(M9):
                    if it > 0:
                        e_b = (
                            e128[:, m]
                            .unsqueeze(1)
                            .broadcast_to([128, IU * US, NU])
                        )
                        nc.vector.tensor_tensor(cw[:, m], wa[:, m], e_b, MUL)
                    for u in range(IU):
                        nc.tensor.matmul(
                            ps[:],
                            xc[:, m, u],
                            rhs_src[:, m, US * u : US * (u + 1)],
                            start=(k == 0),
                            stop=(k == n_mm - 1),
                        )
                        k += 1
                ars = wp.tile([B, US, NU], f32, tag="ars")
                nc.scalar.copy(ars[:], ps[:])
                nc.sync.dma_start(out=ar_in[0:64], in_=ars[:])

                # ---- softmax normalizer partial Z ----
                if it > 0:
                    nc.vector.tensor_reduce(
                        zred[:], e128[:].transpose([0, 2, 1]), mybir.AxisListType.X, ADD
                    )
                    pz = ppz.tile([1, NU], f32, tag="pz")
                    nc.tensor.matmul(pz[:], ones[:], zred[:], start=True, stop=True)
                    nc.vector.tensor_copy(zrow[:, 0:NU], pz[:])
                nc.sync.dma_start(out=ar_in[64:65], in_=zrow[:])

                nc.gpsimd.collective_compute(
                    "AllReduce",
                    ADD,
                    replica_groups=[list(range(N_CORES))],
                    ins=[ar_in.opt()],
                    outs=[ar_out.opt()],
                )

                # keep the PE warm through the AllReduce wait: a chain of
                # gpsimd copies (~2us each) gating dummy matmuls so the HAM
                # activity monitor sees PE work every <3.4us.
                if it < 2:
                    # gpsimd executes in order, and the collective trigger is
                    # a gpsimd instruction — these filler copies pace the
                    # dummy matmuls across the AllReduce wait window.
                    for link in range(7):
                        gc = wp.tile([128, 512], bf16, tag=f"gc{link % 2}")
                        nc.gpsimd.tensor_copy(gc[:], gbig[:])
                        pwk = ppz.tile([2, 2], f32, tag="pz")
                        nc.tensor.matmul(
                            pwk[:], gc[:, 0:2], gc[:, 0:2],
                            start=True, stop=True,
                        )
                    # dense burst: ~4us of back-to-back matmuls flips the
                    # HAM to 2.4GHz right as the AllReduce completes, so the
                    # agreement + next weighted-sum phases run warm
                    pburst = ppz.tile([128, 512], f32, tag="pz")
                    for _ in range(7):
                        nc.tensor.matmul(
                            pburst[:], gbig[:, 0:128], gbig[:],
                            start=True, stop=True,
                        )

                # ---- s = s~/Z, v = squash(s) ----
                s_sb = wp.tile([B, US, NU], f32, tag="s")
                nc.sync.dma_start(out=s_sb[:], in_=ar_out[0:64])
                if it == 0:
                    nc.vector.tensor_scalar_mul(s_sb[:], s_sb[:], 1.0 / IC)
                else:
                    zb1 = wp.tile([1, NU], f32, tag="zb1")
                    nc.sync.dma_start(out=zb1[:], in_=ar_out[64:65, 0:NU])
                    pzb = pps.tile([B, NU], f32, tag="ps")
                    nc.tensor.matmul(pzb[:], ones1[:], zb1[:], start=True, stop=True)
                    rz = wp.tile([B, NU], f32, tag="rz")
                    nc.vector.reciprocal(rz[:], pzb[:])
                    nc.vector.tensor_tensor(
                        s_sb[:], s_sb[:], rz[:].unsqueeze(1).broadcast_to([B, US, NU]), MUL
                    )
                sq = wp.tile([B, US, NU], f32, tag="sq")
                nc.vector.tensor_tensor(sq[:], s_sb[:], s_sb[:], MUL)
                msq = wp.tile([B, US], f32, tag="msq")
                nc.vector.tensor_reduce(msq[:], sq[:], mybir.AxisListType.X, ADD)
                mroot = wp.tile([B, US], f32, tag="mroot")
                nc.scalar.sqrt(mroot[:], msq[:])
                den = wp.tile([B, US], f32, tag="den")
                nc.scalar.activation(
                    den[:], msq[:], mybir.ActivationFunctionType.Identity,
                    bias=(b64 if it < 2 else b1c)[:],
                    scale=float(B) if it < 2 else 1.0,
                )
                if it < 2:
                    nc.scalar.activation(
                        tl2[:], tl1[:], mybir.ActivationFunctionType.Exp
                    )
                rden = wp.tile([B, US], f32, tag="rden")
                nc.vector.reciprocal(rden[:], den[:])
                f_sb = wp.tile([B, US], f32, tag="f")
                nc.vector.tensor_tensor(f_sb[:], mroot[:], rden[:], MUL)

                if it < 2:
                    # ---- agreement update: local u_vj, b += ----
                    f2 = f_sb
                    vB = wp.tile([B, NU, US], bf16, tag="vB")
                    nc.vector.tensor_tensor(
                        vB[:].transpose([0, 2, 1]),
                        s_sb[:],
                        f2[:].unsqueeze(2).broadcast_to([B, US, NU]),
                        MUL,
                    )
                    for m in range(M9):
                        tb = wp.tile([128, IU, SJ], bf16, tag="tb")
                        for h in range(2):
                            pt = ppt.tile([128, 4, 256], f32, tag="pt")
                            for k in range(4):
                                u = 4 * h + k
                                nc.tensor.matmul(
                                    pt[:, k, 0:SJ],
                                    xt[:, u, 128 * m : 128 * (m + 1)],
                                    vB[:],
                                    start=True,
                                    stop=True,
                                )
                            nc.scalar.copy(
                                tb[:, 4 * h : 4 * (h + 1), :], pt[:, :, 0:SJ]
                            )
                        p_sb = wp.tile([128, IU, SJ], bf16, tag="p")
                        nc.vector.tensor_tensor(p_sb[:], wb2[:, m], tb[:], MUL)
                        uvt = b_acc if it == 0 else uv
                        # pairwise add-tree over u (tensor_tensor has a 2x
                        # bf16 uop; tensor_reduce is 1x-only)
                        t1 = wp.tile([128, 4, SJ], bf16, tag="t1")
                        nc.vector.tensor_tensor(
                            t1[:], p_sb[:, 0:4], p_sb[:, 4:8], ADD
                        )
                        nc.vector.tensor_tensor(
                            t1[:, 0:2], t1[:, 0:2], t1[:, 2:4], ADD
                        )
                        nc.vector.tensor_tensor(
                            t1[:, 0], t1[:, 0], t1[:, 1], ADD
                        )
                        nc.vector.tensor_reduce(
                            uvt[:, m],
                            t1[:, 0].rearrange("p (j s) -> p j s", s=US),
                            mybir.AxisListType.X,
                            ADD,
                        )
                        if it > 0:
                            nc.vector.tensor_tensor(
                                b_acc[:, m], b_acc[:, m], uv[:, m], ADD
                            )
                        nc.scalar.activation(
                            e128[:, m],
                            b_acc[:, m],
                            mybir.ActivationFunctionType.Exp,
                        )
                    nc.scalar.sqrt(tl2[:], tl1[:])
                else:
                    # ---- final output v = s * f, stored j-major ----
                    v2 = wp.tile([B, NU, US], f32, tag="v2")
                    nc.vector.tensor_tensor(
                        v2[:].transpose([0, 2, 1]),
                        s_sb[:],
                        f_sb[:].unsqueeze(2).broadcast_to([B, US, NU]),
                        MUL,
                    )
                    nc.sync.dma_start(out=y_out[:], in_=v2[:])
    _split_multi_waits(nc)
    return nc


def _build_warmup_program():
    """Tiny SPMD program with one AllReduce: boots the collectives firmware
    on the TOPSP cores (~65us one-time cost per NRT session) so the main
    kernel's first AllReduce doesn't pay it."""
    from concourse import bass, tile, mybir

    nc = bass.Bass(
        "TRN2", target_bir_lowering=False, debug=False, num_devices=N_CORES
    )
    x_in = nc.dram_tensor("x", [1, 16], mybir.dt.float32, kind="ExternalInput").ap()
    y_out = nc.dram_tensor("y", [1, 16], mybir.dt.float32, kind="ExternalOutput").ap()
    with tile.TileContext(nc) as tc:
        with (
            tc.tile_pool(name="sbuf", bufs=1) as sbuf,
            tc.tile_pool(name="dram", bufs=1, space="DRAM") as dp,
        ):
            t = sbuf.tile([1, 16], mybir.dt.float32)
            nc.sync.dma_start(out=t[:], in_=x_in[:])
            b_in = dp.tile([1, 16], mybir.dt.float32, tag="bi")
            b_out = dp.tile([1, 16], mybir.dt.float32, tag="bo")
            nc.sync.dma_start(out=b_in[:], in_=t[:])
            nc.gpsimd.collective_compute(
                "AllReduce",
                mybir.AluOpType.add,
                replica_groups=[list(range(N_CORES))],
                ins=[b_in.opt()],
                outs=[b_out.opt()],
            )
            nc.sync.dma_start(out=y_out[:], in_=b_out[:])
    _split_multi_waits(nc)
    return nc


def _shard_inputs(x, weight):
    w = np.asarray(weight).reshape(IC, NU, US, IU)
    x = np.asarray(x)
    wb = w.astype(BF16)
    xb = x.astype(BF16)
    in_maps = []
    for c in range(N_CORES):
        i0 = c * S
        ws = wb[i0 : i0 + S]                       # [1152, NU, US, IU]
        wa = np.ascontiguousarray(
            ws.reshape(M9, 128, NU, US, IU).transpose(1, 0, 4, 3, 2)
        ).reshape(128, M9, IU * US, NU)            # [128, 9, u*16+s, NU]
        xs = xb[:, :, i0 : i0 + S]                 # [B, IU, 1152]
        xc = np.ascontiguousarray(
            xs.reshape(B, IU, M9, 128).transpose(3, 2, 1, 0)
        )                                          # [128, 9, IU, B]
        xt = np.ascontiguousarray(xs)              # [B, IU, 1152]
        wb2 = np.ascontiguousarray(
            ws.reshape(M9, 128, NU, US, IU).transpose(1, 0, 4, 2, 3)
        ).reshape(128, M9, IU, SJ)                 # [128, 9, u, j*16+s]
        in_maps.append({"wa": wa, "xc": xc, "xt": xt, "wb": wb2})
    return in_maps


def kernel(x, weight):
    from concourse.bass_utils import run_bass_kernel_spmd

    if "nc" not in _CACHE:
        _CACHE["nc"] = _build_program()
    in_maps = _shard_inputs(x, weight)
    res = run_bass_kernel_spmd(_CACHE["nc"], in_maps, list(range(N_CORES)))
    y = np.asarray(res.results[0]["y"], dtype=np.float32)
    return y.reshape(B, NU, US, 1)

